# revision 2
# baseline (speedup 1.0000x reference)
"""Trainium2 Bass kernel for nn_GAT_15994458210581.

Pipeline: BatchNorm(train) -> GATConv(8 heads, concat=False/mean, self-loops)
-> ELU -> global_mean_pool over 100 graphs.

Strategy (8 NeuronCores, SPMD):
- Nodes (and their incident edges) are partitioned across cores by dst range.
- Phase A (replicated): BN stats + h = xn@W + attention terms; each core
  writes a full node table row [h(128) | a_src(8) | a_dst(8) | pad] (768B)
  to its own DRAM.
- Phase B (per core): edges sorted by (dst-block, src-segment); h[src] rows
  gathered with dma_gather (int16 segment-local indices); attention softmax
  (exp without max-subtraction - logits are bounded) and the segment-sum are
  done with one-hot matmuls accumulating per-dst-block PSUM.
- Per-node out -> ELU -> per-graph partial sums via one-hot matmul; host sums
  the per-core partials (graph boundary overlap) and divides by graph counts.
"""
import sys

sys.path.insert(0, "/opt/trn_rl_repo")
import numpy as np

EPS = 1e-5
NEG_SLOPE = 0.2


def _default_cfg():
    return dict(N=100000, F=128, H=8, C=16, G=100, NCORES=8, SEG=32768, MAXCH=8)


def _derive(cfg):
    d = dict(cfg)
    d["HC"] = d["H"] * d["C"]
    d["NT"] = -(-d["N"] // 128)
    d["TROWS"] = d["NT"] * 128
    d["NPC"] = -(-d["N"] // d["NCORES"] // 128) * 128
    d["B"] = d["NPC"] // 128
    d["NSEG"] = -(-d["TROWS"] // d["SEG"])
    d["ROWF"] = d["F"] + 64          # 128 h + 8 a_src + 8 a_dst + 40 pad
    d["AOFF"] = d["F"]
    d["ADOFF"] = d["F"] + 8
    assert d["ROWF"] * 4 % 256 == 0
    return d


def host_prep(cfg, edge_index, batch):
    """Shard/sort/pad edges; build per-core streams and the uniform structure."""
    c = cfg
    N, NC, NPC, B, SEG, NSEG = c["N"], c["NCORES"], c["NPC"], c["B"], c["SEG"], c["NSEG"]
    MAXCH = c["MAXCH"]

    src = np.concatenate([np.asarray(edge_index[0]), np.arange(N)]).astype(np.int64)
    dst = np.concatenate([np.asarray(edge_index[1]), np.arange(N)]).astype(np.int64)
    batch = np.asarray(batch).astype(np.int64)

    core = np.minimum(dst // NPC, NC - 1)
    dloc = dst - core * NPC
    blk = dloc // 128
    seg = src // SEG

    cnt = np.zeros((NC, B, NSEG), np.int64)
    np.add.at(cnt, (core, blk, seg), 1)
    mx = cnt.max(axis=0)
    cnt16 = -(-mx // 16) * 16

    structure = []                       # (b, s, cnt16_piece)
    for b in range(B):
        for s in range(NSEG):
            left = int(cnt16[b, s])
            while left > 0:
                piece = min(left, MAXCH * 128)
                structure.append((b, s, piece))
                left -= piece

    coloff, choff = [], []
    o_c = o_ch = 0
    for (b, s, c16) in structure:
        coloff.append(o_c); choff.append(o_ch)
        o_c += c16 // 16
        o_ch += -(-c16 // 128)
    TOTCOLS, TOTCH = max(o_c, 1), max(o_ch, 1)

    # flat row-stream: group gi occupies [rowoff[gi], rowoff[gi]+nch*128);
    # staged to SBUF in batches of STAGE_G groups
    STAGE_G = 8
    rowoff = []
    o_r = 0
    batch_of = []
    stage_start = []            # per batch: (stream offset, length)
    for gi, (b, s, c16) in enumerate(structure):
        if gi % STAGE_G == 0:
            stage_start.append([o_r, 0])
        batch_of.append(len(stage_start) - 1)
        rowoff.append(o_r)
        o_r += -(-c16 // 128) * 128
        stage_start[-1][1] = o_r - stage_start[-1][0]
    TOTROW = max(o_r, 1)
    SCAP = max((s[1] for s in stage_start), default=1)

    order = np.lexsort((seg, blk, core))
    src_s, blk_s, seg_s, dloc_s = src[order], blk[order], seg[order], dloc[order]
    core_s = core[order]
    key = (core_s * B + blk_s) * NSEG + seg_s
    kk = np.arange(NC * B * NSEG)
    starts = np.searchsorted(key, kk)
    ends = np.searchsorted(key, kk + 1)

    idx16 = np.zeros((NC, 128, TOTCOLS), np.int16)
    drel_col = np.full((NC, 128, TOTCH), 999.0, np.float32)
    drel_row = np.full((NC, 1, TOTROW), 999.0, np.float32)

    consumed = {}
    for gi, (b, s, c16) in enumerate(structure):
        nch = -(-c16 // 128)
        for m in range(NC):
            k = (m * B + b) * NSEG + s
            st, en = starts[k], ends[k]
            already = consumed.get((m, b, s), 0)
            take = max(0, min(en - st - already, c16))
            lo = st + already
            consumed[(m, b, s)] = already + take
            idxs = np.zeros(c16, np.int16)
            drels = np.full(nch * 128, 999.0, np.float32)
            if take > 0:
                idxs[:take] = (src_s[lo:lo + take] - s * SEG).astype(np.int16)
                drels[:take] = (dloc_s[lo:lo + take] - b * 128).astype(np.float32)
            wrapped = idxs.reshape(c16 // 16, 16).T
            idx16[m, :, coloff[gi]:coloff[gi] + c16 // 16] = np.tile(wrapped, (8, 1))
            dr = drels.reshape(nch, 128)
            drel_col[m, :, choff[gi]:choff[gi] + nch] = dr.T
            drel_row[m, 0, rowoff[gi]:rowoff[gi] + nch * 128] = drels

    batchrel = np.full((NC, 128, B), 999.0, np.float32)
    blocknode = np.zeros((NC, 128, B), np.int32)
    g0 = np.zeros(NC, np.int64)
    for m in range(NC):
        lo = m * NPC
        hi = min(lo + NPC, N)
        n = hi - lo
        g0[m] = batch[min(lo, N - 1)]
        br = np.full(NPC, 999.0, np.float32)
        bn = np.zeros(NPC, np.int32)
        if n > 0:
            br[:n] = (batch[lo:hi] - g0[m]).astype(np.float32)
            bn[:n] = np.arange(lo, hi, dtype=np.int32)
        batchrel[m] = br.reshape(B, 128).T
        blocknode[m] = bn.reshape(B, 128).T

    return dict(structure=structure, coloff=coloff, choff=choff,
                rowoff=rowoff, batch_of=batch_of, stage_start=stage_start,
                STAGE_G=STAGE_G, TOTCOLS=TOTCOLS, TOTCH=TOTCH, TOTROW=TOTROW,
                SCAP=SCAP, idx16=idx16, drel_col=drel_col, drel_row=drel_row,
                batchrel=batchrel, blocknode=blocknode, g0=g0)


def build_nc(cfg, prep, num_devices, variant='full'):
    import concourse.bass as bass
    import concourse.bacc as bacc
    import concourse.mybir as mybir
    from concourse.tile import TileContext
    from contextlib import ExitStack

    c = cfg
    f32 = mybir.dt.float32
    i16 = mybir.dt.int16
    i32 = mybir.dt.int32
    Alu = mybir.AluOpType
    Act = mybir.ActivationFunctionType
    N, F, HC, H, Cc = c["N"], c["F"], c["HC"], c["H"], c["C"]
    NT, TROWS, B, SEG, NSEG = c["NT"], c["TROWS"], c["B"], c["SEG"], c["NSEG"]
    ROWF, AOFF, ADOFF, MAXCH = c["ROWF"], c["AOFF"], c["ADOFF"], c["MAXCH"]
    structure, coloff, choff = prep["structure"], prep["coloff"], prep["choff"]
    rowoff, batch_of, stage_start = prep["rowoff"], prep["batch_of"], prep["stage_start"]
    STAGE_G = prep["STAGE_G"]
    TOTCOLS, TOTCH, TOTROW, SCAP = (prep["TOTCOLS"], prep["TOTCH"],
                                    prep["TOTROW"], prep["SCAP"])
    SUB = 4 * 128                      # onehotN sub-slice (PSUM free-dim cap)

    nc = bacc.Bacc("TRN2", target_bir_lowering=False, debug=False,
                   num_devices=num_devices)
    x_d = nc.dram_tensor("x", [N, F], f32, kind="ExternalInput")
    W_d = nc.dram_tensor("W", [F, HC], f32, kind="ExternalInput")
    gamma_d = nc.dram_tensor("gamma", [F, 1], f32, kind="ExternalInput")
    beta_d = nc.dram_tensor("beta", [F, 1], f32, kind="ExternalInput")
    attb_d = nc.dram_tensor("attboth", [HC, 16], f32, kind="ExternalInput")
    biasm_d = nc.dram_tensor("bias_mat", [128, Cc], f32, kind="ExternalInput")
    ident_d = nc.dram_tensor("ident", [128, 128], f32, kind="ExternalInput")
    iotac_d = nc.dram_tensor("iota_col", [128, 1], f32, kind="ExternalInput")
    iotam_d = nc.dram_tensor("iota_mat", [128, 128], f32, kind="ExternalInput")
    idx_d = nc.dram_tensor("idx16", [128, TOTCOLS], i16, kind="ExternalInput")
    drc_d = nc.dram_tensor("drel_col", [128, TOTCH], f32, kind="ExternalInput")
    drr_d = nc.dram_tensor("drel_row", [1, TOTROW], f32, kind="ExternalInput")
    brel_d = nc.dram_tensor("batchrel", [128, B], f32, kind="ExternalInput")
    bnode_d = nc.dram_tensor("blocknode", [128, B], i32, kind="ExternalInput")
    out_d = nc.dram_tensor("pool_out", [128, Cc], f32, kind="ExternalOutput")
    table = nc.dram_tensor("table", [TROWS, ROWF], f32)

    segrows = [min(SEG, TROWS - s * SEG) for s in range(NSEG)]

    with TileContext(nc) as tc, ExitStack() as ctx:
        cp = ctx.enter_context(tc.tile_pool(name="consts", bufs=1))

        def cload(name, dram, shape, dt=f32):
            t = cp.tile(shape, dt, tag=name)
            nc.sync.dma_start(out=t[:], in_=dram[:, :])
            return t

        W_t = cload("W", W_d, [F, HC])
        gamma_t = cload("gam", gamma_d, [F, 1])
        beta_t = cload("bet", beta_d, [F, 1])
        attb_t = cload("attb", attb_d, [HC, 16])
        biasm_t = cload("biasm", biasm_d, [128, Cc])
        ident_t = cload("ident", ident_d, [128, 128])
        iotac_t = cload("iotac", iotac_d, [128, 1])
        iotam_t = cload("iotam", iotam_d, [128, 128])
        idx_t = cload("idx", idx_d, [128, TOTCOLS], i16)
        drc_t = cload("drc", drc_d, [128, TOTCH])
        brel_t = cload("brel", brel_d, [128, B])
        bnode_t = cload("bnode", bnode_d, [128, B], i32)
        ones_t = cp.tile([128, 1], f32, tag="ones")
        nc.vector.memset(ones_t[:], 1.0)
        ones1_t = cp.tile([1, 128], f32, tag="ones1")
        nc.vector.memset(ones1_t[:], 1.0)
        mean_t = cp.tile([F, 1], f32, tag="mean")
        scale_t = cp.tile([F, 1], f32, tag="scl")
        shift_t = cp.tile([F, 1], f32, tag="shf")
        Wp_t = cp.tile([F, HC], f32, tag="Wp")
        c0_t = cp.tile([HC, 1], f32, tag="c0")

        # ---------------- Phase A pass 1: BN stats ----------------
        with tc.tile_pool(name="p1", bufs=4) as p1, \
             tc.tile_pool(name="p1ps", bufs=1, space="PSUM") as p1ps:
            statx = p1ps.tile([F, 1], f32, tag="sx")
            statx2 = p1ps.tile([F, 1], f32, tag="sx2")
            for t in range(NT):
                xt = p1.tile([128, F], f32, tag="x")
                rows = min(128, N - t * 128)
                if rows < 128:
                    nc.vector.memset(xt[:], 0.0)
                nc.sync.dma_start(out=xt[:rows, :], in_=x_d[t * 128:t * 128 + rows, :])
                sq = p1.tile([128, F], f32, tag="sq")
                nc.vector.tensor_tensor(out=sq[:], in0=xt[:], in1=xt[:], op=Alu.mult)
                nc.tensor.matmul(out=statx[:], lhsT=xt[:], rhs=ones_t[:],
                                 start=(t == 0), stop=(t == NT - 1))
                nc.tensor.matmul(out=statx2[:], lhsT=sq[:], rhs=ones_t[:],
                                 start=(t == 0), stop=(t == NT - 1))
            tmp = p1.tile([F, 1], f32, tag="tmp")
            tmp2 = p1.tile([F, 1], f32, tag="tmp2")
            nc.vector.tensor_scalar_mul(out=mean_t[:], in0=statx[:], scalar1=1.0 / N)
            nc.vector.tensor_scalar_mul(out=tmp[:], in0=statx2[:], scalar1=1.0 / N)
            nc.vector.tensor_tensor(out=tmp2[:], in0=mean_t[:], in1=mean_t[:], op=Alu.mult)
            nc.vector.tensor_tensor(out=tmp[:], in0=tmp[:], in1=tmp2[:], op=Alu.subtract)
            nc.vector.tensor_scalar_add(out=tmp[:], in0=tmp[:], scalar1=EPS)
            nc.scalar.activation(out=tmp[:], in_=tmp[:], func=Act.Sqrt)
            nc.vector.reciprocal(out=tmp2[:], in_=tmp[:])
            nc.vector.tensor_tensor(out=scale_t[:], in0=tmp2[:], in1=gamma_t[:], op=Alu.mult)
            nc.vector.tensor_tensor(out=tmp[:], in0=mean_t[:], in1=scale_t[:], op=Alu.mult)
            nc.vector.tensor_tensor(out=shift_t[:], in0=beta_t[:], in1=tmp[:], op=Alu.subtract)
            nc.vector.tensor_scalar_mul(out=Wp_t[:], in0=W_t[:], scalar1=scale_t[:, 0:1])
            c0ps = p1ps.tile([HC, 1], f32, tag="c0p")
            nc.tensor.matmul(out=c0ps[:], lhsT=W_t[:], rhs=shift_t[:], start=True, stop=True)
            nc.vector.tensor_copy(out=c0_t[:], in_=c0ps[:])

        # ---------------- Phase A pass 2: node table ----------------
        with tc.tile_pool(name="p2", bufs=3) as p2, \
             tc.tile_pool(name="p2ps", bufs=2, space="PSUM") as p2ps:
            for t in range(NT if variant != 'a1' else 0):
                xt = p2.tile([128, F], f32, tag="x2")
                rows = min(128, N - t * 128)
                if rows < 128:
                    nc.vector.memset(xt[:], 0.0)
                nc.sync.dma_start(out=xt[:rows, :], in_=x_d[t * 128:t * 128 + rows, :])
                xTp = p2ps.tile([F, 128], f32, tag="xT")
                nc.tensor.transpose(out=xTp[:], in_=xt[:], identity=ident_t[:])
                xT = p2.tile([F, 128], f32, tag="xTs")
                nc.scalar.activation(out=xT[:], in_=xTp[:], func=Act.Copy)
                hTp = p2ps.tile([HC, 128], f32, tag="hT")
                nc.tensor.matmul(out=hTp[:], lhsT=Wp_t[:], rhs=xT[:], start=True, stop=True)
                hT = p2.tile([HC, 128], f32, tag="hTs")
                nc.scalar.activation(out=hT[:], in_=hTp[:], func=Act.Identity, bias=c0_t[:, 0:1])
                hp = p2ps.tile([128, HC], f32, tag="h")
                nc.tensor.transpose(out=hp[:], in_=hT[:], identity=ident_t[:])
                ap_ = p2ps.tile([128, 16], f32, tag="a")
                nc.tensor.matmul(out=ap_[:], lhsT=hT[:], rhs=attb_t[:], start=True, stop=True)
                row = p2.tile([128, ROWF], f32, tag="row")
                nc.vector.tensor_copy(out=row[:, 0:HC], in_=hp[:])
                nc.vector.tensor_copy(out=row[:, AOFF:AOFF + 16], in_=ap_[:])
                nc.vector.memset(row[:, AOFF + 16:ROWF], 0.0)
                nc.sync.dma_start(out=table[t * 128:(t + 1) * 128, :], in_=row[:])

        # ---------------- Phase B ----------------
        gb = ctx.enter_context(tc.tile_pool(name="g", bufs=3))
        stg = ctx.enter_context(tc.tile_pool(name="stg", bufs=2))
        onp = ctx.enter_context(tc.tile_pool(name="onN", bufs=2))
        oep = ctx.enter_context(tc.tile_pool(name="onE", bufs=3))
        rp = ctx.enter_context(tc.tile_pool(name="rhs", bufs=3))
        wp = ctx.enter_context(tc.tile_pool(name="wts", bufs=2))
        adw_p = ctx.enter_context(tc.tile_pool(name="adw", bufs=2))
        pp = ctx.enter_context(tc.tile_pool(name="post", bufs=2))
        ups = ctx.enter_context(tc.tile_pool(name="ups", bufs=2, space="PSUM"))
        aps = ctx.enter_context(tc.tile_pool(name="aps", bufs=2, space="PSUM"))
        dbp = ctx.enter_context(tc.tile_pool(name="dbp", bufs=2, space="PSUM"))
        gps = ctx.enter_context(tc.tile_pool(name="gps", bufs=1, space="PSUM"))

        pool_ps = gps.tile([128, Cc], f32, tag="pool")

        cur_batch = [-1]
        cur_stage = [None]
        cur_soff = [0]
        by_block = [[] for _ in range(B)]
        for gi, (b, s, c16) in enumerate(structure):
            by_block[b].append((gi, s, c16))

        for b in range(B if variant not in ('a1', 'a12') else 0):
            adw = adw_p.tile([128, 8], f32, tag="adw")
            nc.gpsimd.indirect_dma_start(
                out=adw[:], out_offset=None, in_=table[:, :],
                in_offset=bass.IndirectOffsetOnAxis(ap=bnode_t[:, b:b + 1], axis=0),
                element_offset=ADOFF)
            u_ps = ups.tile([128, HC + 8], f32, tag="u")
            nch_b = sum(-(-c16 // 128) for (_, _, c16) in by_block[b])
            ci = 0
            for (gi, s, c16) in by_block[b]:
                nch = -(-c16 // 128)
                g = gb.tile([128, MAXCH, ROWF], f32, tag="g")
                if c16 < nch * 128:
                    nc.vector.memset(g[:, nch - 1, :], 0.0)
                nc.gpsimd.dma_gather(
                    out_ap=g[:, 0:nch, :],
                    in_ap=table[s * SEG:s * SEG + segrows[s], :],
                    idxs_ap=idx_t[:, coloff[gi]:coloff[gi] + c16 // 16],
                    num_idxs=c16, num_idxs_reg=c16, elem_size=ROWF,
                    single_packet=False)
                if batch_of[gi] != cur_batch[0]:
                    cur_batch[0] = batch_of[gi]
                    soff, slen = stage_start[cur_batch[0]]
                    st_t = stg.tile([1, SCAP], f32, tag="stage")
                    nc.sync.dma_start(out=st_t[0:1, 0:slen],
                                      in_=drr_d[0:1, soff:soff + slen])
                    cur_stage[0] = st_t
                    cur_soff[0] = soff
                st_t = cur_stage[0]
                roff = rowoff[gi] - cur_soff[0]
                onN = onp.tile([128, MAXCH * 128], f32, tag="onN")
                for k in range(0, nch * 128, SUB):
                    L = min(SUB, nch * 128 - k)
                    dbc = dbp.tile([128, SUB], f32, tag="dbc")
                    nc.tensor.matmul(out=dbc[:, 0:L], lhsT=ones1_t[:],
                                     rhs=st_t[0:1, roff + k:roff + k + L],
                                     start=True, stop=True)
                    nc.vector.tensor_tensor(
                        out=onN[:, k:k + L],
                        in0=iotac_t[:, 0:1].to_broadcast([128, L]),
                        in1=dbc[:, 0:L], op=Alu.is_equal)
                egrp = wp.tile([128, MAXCH * 8], f32, tag="egrp")
                for cch in range(nch):
                    ae = aps.tile([128, 8], f32, tag="ae")
                    nc.tensor.matmul(out=ae[:], lhsT=onN[:, cch * 128:(cch + 1) * 128],
                                     rhs=adw[:], start=True, stop=True)
                    nc.vector.tensor_tensor(out=egrp[:, cch * 8:(cch + 1) * 8],
                                            in0=g[:, cch, AOFF:AOFF + 8],
                                            in1=ae[:], op=Alu.add)
                t1 = wp.tile([128, MAXCH * 8], f32, tag="t1")
                nc.vector.tensor_scalar_mul(out=t1[:, 0:nch * 8], in0=egrp[:, 0:nch * 8],
                                            scalar1=NEG_SLOPE)
                nc.vector.tensor_tensor(out=t1[:, 0:nch * 8], in0=t1[:, 0:nch * 8],
                                        in1=egrp[:, 0:nch * 8], op=Alu.max)
                wt = wp.tile([128, MAXCH * 8], f32, tag="w")
                nc.scalar.activation(out=wt[:, 0:nch * 8], in_=t1[:, 0:nch * 8], func=Act.Exp)
                for cch in range(nch):
                    rhs_t = rp.tile([128, HC + 8], f32, tag="rhs")
                    wb = wt[:, cch * 8:(cch + 1) * 8]
                    nc.vector.tensor_tensor(
                        out=rhs_t[:, 0:HC].rearrange("p (h c2) -> p h c2", h=H),
                        in0=g[:, cch, 0:HC].rearrange("p (h c2) -> p h c2", h=H),
                        in1=wb.to_broadcast([128, 8, Cc]), op=Alu.mult)
                    nc.vector.tensor_copy(out=rhs_t[:, HC:HC + 8], in_=wb)
                    if variant == 'nomsg':
                        ci += 1
                        continue
                    onE = oep.tile([128, 128], f32, tag="onE")
                    nc.vector.tensor_tensor(
                        out=onE[:],
                        in0=drc_t[:, choff[gi] + cch:choff[gi] + cch + 1].to_broadcast([128, 128]),
                        in1=iotam_t[:], op=Alu.is_equal)
                    nc.tensor.matmul(out=u_ps[:], lhsT=onE[:], rhs=rhs_t[:],
                                     start=(ci == 0), stop=(ci == nch_b - 1))
                    ci += 1
            # ---- postprocess block ----
            s_sb = pp.tile([128, 8], f32, tag="s")
            nc.vector.tensor_scalar_add(out=s_sb[:], in0=u_ps[:, HC:HC + 8], scalar1=1e-30)
            rs = pp.tile([128, 8], f32, tag="rs")
            nc.vector.reciprocal(out=rs[:], in_=s_sb[:])
            prod = pp.tile([128, HC], f32, tag="prod")
            nc.vector.tensor_tensor(
                out=prod[:].rearrange("p (h c2) -> p h c2", h=H),
                in0=u_ps[:, 0:HC].rearrange("p (h c2) -> p h c2", h=H),
                in1=rs[:].to_broadcast([128, 8, Cc]), op=Alu.mult)
            o16 = pp.tile([128, Cc], f32, tag="o16")
            nc.vector.tensor_reduce(out=o16[:], in_=prod[:].rearrange("p (h c2) -> p c2 h", h=H),
                                    axis=mybir.AxisListType.X, op=Alu.add)
            nc.vector.tensor_scalar_mul(out=o16[:], in0=o16[:], scalar1=1.0 / H)
            nc.vector.tensor_tensor(out=o16[:], in0=o16[:], in1=biasm_t[:], op=Alu.add)
            mask = pp.tile([128, Cc], mybir.dt.uint8, tag="mask")
            nc.vector.tensor_scalar(out=mask[:], in0=o16[:], scalar1=0.0, scalar2=None,
                                    op0=Alu.is_gt)
            ex = pp.tile([128, Cc], f32, tag="ex")
            nc.scalar.activation(out=ex[:], in_=o16[:], func=Act.Exp)
            nc.vector.tensor_scalar_add(out=ex[:], in0=ex[:], scalar1=-1.0)
            onode = pp.tile([128, Cc], f32, tag="onode")
            nc.vector.tensor_copy(out=onode[:], in_=ex[:])
            nc.vector.copy_predicated(out=onode[:], mask=mask[:], data=o16[:])
            onG = oep.tile([128, 128], f32, tag="onG")
            nc.vector.tensor_tensor(
                out=onG[:],
                in0=brel_t[:, b:b + 1].to_broadcast([128, 128]),
                in1=iotam_t[:], op=Alu.is_equal)
            nc.tensor.matmul(out=pool_ps[:], lhsT=onG[:], rhs=onode[:],
                             start=(b == 0), stop=(b == B - 1))

        if variant not in ('a1', 'a12'):
            outp_t = pp.tile([128, Cc], f32, tag="out")
            nc.vector.tensor_copy(out=outp_t[:], in_=pool_ps[:])
            nc.sync.dma_start(out=out_d[:, :], in_=outp_t[:])

    nc.compile()
    return nc


def _np_f32(a):
    return np.ascontiguousarray(np.asarray(a), dtype=np.float32)


def make_in_maps(cfg, prep, inputs):
    c = cfg
    F, H, Cc, HC, NC = c["F"], c["H"], c["C"], c["HC"], c["NCORES"]
    x = _np_f32(inputs["x"])
    W = _np_f32(inputs["W"])
    gamma = _np_f32(inputs["bn_gamma"]).reshape(F, 1)
    beta = _np_f32(inputs["bn_beta"]).reshape(F, 1)
    att_src = _np_f32(inputs["att_src"])
    att_dst = _np_f32(inputs["att_dst"])
    bias = _np_f32(inputs["bias"]).reshape(1, Cc)

    attboth = np.zeros((HC, 16), np.float32)
    for h in range(H):
        attboth[h * Cc:(h + 1) * Cc, h] = att_src[h]
        attboth[h * Cc:(h + 1) * Cc, 8 + h] = att_dst[h]
    shared = dict(
        x=x, W=W, gamma=gamma, beta=beta, attboth=attboth,
        bias_mat=np.tile(bias, (128, 1)),
        ident=np.eye(128, dtype=np.float32),
        iota_col=np.arange(128, dtype=np.float32).reshape(128, 1),
        iota_mat=np.tile(np.arange(128, dtype=np.float32), (128, 1)),
    )
    in_maps = []
    for m in range(NC):
        im = dict(shared)
        im["idx16"] = prep["idx16"][m]
        im["drel_col"] = prep["drel_col"][m]
        im["drel_row"] = prep["drel_row"][m]
        im["batchrel"] = prep["batchrel"][m]
        im["blocknode"] = prep["blocknode"][m]
        in_maps.append(im)
    return in_maps


def unshard(cfg, prep, results):
    c = cfg
    G, Cc, NC = c["G"], c["C"], c["NCORES"]
    batchcnt = prep["graph_counts"]
    out = np.zeros((G, Cc), np.float64)
    for m in range(NC):
        pool_m = results[m]["pool_out"]
        g0 = int(prep["g0"][m])
        hi = min(128, G - g0)
        out[g0:g0 + hi] += pool_m[:hi]
    out = out / np.maximum(batchcnt, 1.0)[:, None]
    return out.astype(np.float32)


_CACHE = {}
LAST_RESULT = None


def kernel(**inputs):
    import os
    from concourse.bass_utils import run_bass_kernel_spmd

    cfg = _derive(_default_cfg())
    batch = np.asarray(inputs["batch"]).astype(np.int64)
    prep = host_prep(cfg, inputs["edge_index"], batch)
    prep["graph_counts"] = np.bincount(batch, minlength=cfg["G"]).astype(np.float64)
    key = "full"
    if key not in _CACHE:
        _CACHE[key] = build_nc(cfg, prep, cfg["NCORES"])
    nc = _CACHE[key]
    in_maps = make_in_maps(cfg, prep, inputs)
    kw = {}
    if os.environ.get("KERNEL_TRACE"):
        kw["trace"] = True
        if os.environ.get("KERNEL_TRACE_DIR"):
            kw["tmpdir"] = os.environ["KERNEL_TRACE_DIR"]
    res = run_bass_kernel_spmd(nc, in_maps, list(range(cfg["NCORES"])), **kw)
    global LAST_RESULT
    LAST_RESULT = res
    return unshard(cfg, prep, res.results)

